# revision 13
# baseline (speedup 1.0000x reference)
"""ARX recurrence kernel for Trainium2 (8 NeuronCores, data-parallel).

Math: the reference runs out[:, t] = window @ w_ar + (u @ w_u + w_b) as a
sequential scan over 1008 steps.  Since the recurrence is linear, every
output timestep is a linear functional of X = [y | u | 1]:

    out[:, t] = X @ G[:, t]          G: [32, 1024]

G depends only on the 32-element weight vector, so it is computed on the
HOST in float64 and shipped as a small bf16 input, pre-replicated across
the 4 partition quadrants.  out[:, :16] is just y, so the device only
computes/stores the 1008 predicted columns; the host splices y back in.

v2 changes vs the fp32 baseline (85.7 us) -> ~27-30 us/iter measured:
  * bf16 X^T / G inputs and bf16 output stores: the dominant output
    traffic drops 32 MB -> 15.75 MB per core.  Error ~2.4e-3 norm-rel
    vs the 2e-2 gate (PE accumulates in fp32; PSUM is fp32).
  * X^T is packed on the HOST into the exact [128, 32*ntiles] lhsT
    layout (row 32j+k, col 128g+q), removing the on-device PE
    transposes, identity matrix, and gpsimd X4 assembly.
  * Matmuls issue in band-major waves so the 4 row-tiled (K=32)
    matmuls run CONCURRENTLY on the PE 32-row sub-arrays.
  * Drains (PSUM fp32 -> SBUF bf16 cast) are one FD=1008 copy per
    band, split between DVE and ACT by a greedy balance of their
    errata-model per-instruction costs.  The steady state is
    DRAIN-bound: DVE+ACT are the only PSUM-reading engines (~1
    col/cyc each); raw store bandwidth measured ~950 GB/s (DMA_ONLY
    ablation 16.7 us), PE ~10 us, so neither binds.
  * Stores ride the SP HWDGE ring ONLY: a store's sem-wait queued on
    the ACT ring blocks ACT's subsequent drain copies behind DVE's
    progress (measured 34.4 -> 27.2 us).  SWDGE (gpsimd) stores
    measured far worse (87 us).  Store descriptor size 1-4 KB and
    OUT_SPAN made no measurable difference.

Batch <-> partition mapping: partition q holds batch rows
[64*q, 64*q + 64); batch tile s = {64q + s} maps to PE column q, so the
output store is contiguous per partition, chopped into ~4 KB
descriptors, alternating between the SP and ACT HWDGE rings.
"""

import numpy as np

import concourse.bacc as bacc
import concourse.bass as bass
import concourse.mybir as mybir
import concourse.tile as tile
from concourse.bass_utils import run_bass_kernel_spmd

N_CORES = 8
B_FULL = 65536
AR = 16          # ar order
NU = 15          # exogenous dim
K = 32           # regressor dim = AR + NU + 1
S = 1024         # sequence length
SP = S - AR      # 1008 predicted columns actually computed on device

B = B_FULL // N_CORES      # 8192 rows per core
NTILES = B // 128          # 64 batch tiles of 128 rows
GROUPS = NTILES // 4       # 16 groups of 4 tiles
F32 = mybir.dt.float32
BF16 = mybir.dt.bfloat16
NPBF16 = mybir.dt.np(BF16)

# scheduling knobs
OUT_BUFS = 4
PS_BUFS = 4             # [128,1024] fp32 tiles = 2 PSUM banks each
XT_CHUNKS = 8           # X^T loaded in chunks for pipelining
OUT_SPAN = 1            # groups per output store (1 -> ~1MB bf16)
OUT_NDESC = 2           # descriptors per partition per group-span
# stores on the SP ring ONLY: a store's sem-wait in the ACT queue blocks
# ACT's subsequent drains behind DVE's progress (measured 34.4 -> 27.2us)
STORE_RINGS = ("sync",)
DMA_ONLY = False        # ablation: only output stores (garbage data)
DO_MM = True            # ablation: skip matmuls
DO_STORE = True         # ablation: skip output stores (drain-floor probe)
# greedy drain balance: estimated per-drain cost (ns) on each engine
DVE_COST = (120 + 1008) / 0.96   # errata-adjusted PSUM->SBUF, 0.96 GHz
ACT_COST = (172 + 1008) / 1.2
GP_COST = None                   # ns/drain on gpsimd; None disables


def host_g(w):
    """Compute G [32, S] on the host in float64 (cols 0:AR = identity)."""
    w = np.asarray(w, np.float64)
    w_ar, w_u, w_b = w[:AR], w[AR : AR + NU], w[AR + NU]
    Wc = np.zeros((AR, AR + 1))
    Wc[:, :AR] = np.eye(AR)
    preds = np.empty((SP, AR + 1))
    for t in range(SP):
        pc = w_ar @ Wc
        pc[AR] += 1.0
        preds[t] = pc
        Wc = np.concatenate([Wc[1:], pc[None, :]], axis=0)
    G = np.zeros((K, S), np.float64)
    G[:AR, :AR] = np.eye(AR)
    G[:AR, AR:] = preds[:, :AR].T
    G[AR : AR + NU, AR:] = np.outer(w_u, preds[:, AR])
    G[K - 1, AR:] = w_b * preds[:, AR]
    return G.astype(np.float32)


def build_nc(b=B, reps=1):
    """Build the per-core Bass program (SPMD: same program, 8 shards).

    reps>1 unrolls the whole main loop multiple times inside one NEFF
    (writes the same outputs each rep) — used only for steady-state HW
    timing, never for grading."""
    ntiles = b // 128
    groups = ntiles // 4

    nc = bacc.Bacc("TRN2", target_bir_lowering=False, debug=False)

    xt_d = nc.dram_tensor("xt", [128, ntiles * K], BF16,
                          kind="ExternalInput").ap()
    g_d = nc.dram_tensor("g", [128, SP], BF16, kind="ExternalInput").ap()
    out_d = nc.dram_tensor("out", [128, ntiles * SP], BF16,
                           kind="ExternalOutput").ap()

    gcols = 4 * SP           # bf16 out columns per group
    n2 = SP - 512            # second matmul free dim (496)

    from contextlib import ExitStack
    with tile.TileContext(nc) as tc, ExitStack() as ctx:
        singles = ctx.enter_context(tc.tile_pool(name="singles", bufs=1))
        out_pool = ctx.enter_context(tc.tile_pool(name="outsb", bufs=OUT_BUFS))
        ps_pool = ctx.enter_context(
            tc.tile_pool(name="ps", bufs=PS_BUFS, space="PSUM"))

        # G (bf16, 1008 predicted cols), pre-replicated across quadrants.
        # Loaded in the two halves the matmul waves consume, so wave h0
        # only waits for the first 512 columns.
        G_rep = singles.tile([128, SP], BF16, tag="Grep")
        nc.sync.dma_start(out=G_rep[:, 0:512], in_=g_d[:, 0:512])
        nc.sync.dma_start(out=G_rep[:, 512:SP], in_=g_d[:, 512:SP])

        # X^T in lhsT layout, loaded in chunks so group 0 starts early.
        xt_sb = singles.tile([128, ntiles * K], BF16, tag="xt")
        nchunks = max(1, min(XT_CHUNKS, groups))
        ccols = ntiles * K // nchunks
        for c in range(nchunks):
            nc.scalar.dma_start(
                out=xt_sb[:, c * ccols : (c + 1) * ccols],
                in_=xt_d[:, c * ccols : (c + 1) * ccols])

        t_dve = t_act = t_gp = 0.0  # virtual clocks for greedy drain balance
        for g in [g for _ in range(reps) for g in range(groups)]:
            sp = g % OUT_SPAN
            if sp == 0:
                out_sb = out_pool.tile([128, OUT_SPAN * gcols], BF16,
                                       tag="outsb")
            base = sp * gcols

            if not DMA_ONLY:
                # 2 band-major waves of 4 concurrent row-tiled matmuls
                pss = []
                for j in range(4):
                    ps = ps_pool.tile([128, 1024], F32, tag="ps", name="ps")
                    pss.append(ps)
                if DO_MM:
                    for c0, nn in ((0, 512), (512, n2)):
                        for j in range(4):
                            nc.tensor.matmul(
                                pss[j][:, c0 : c0 + nn],
                                xt_sb[32 * j : 32 * (j + 1),
                                      128 * g : 128 * (g + 1)],
                                G_rep[32 * j : 32 * (j + 1), c0 : c0 + nn],
                                start=True, stop=True,
                                tile_position=(32 * j, 0),
                            )
                # drains: one fp32->bf16 FD=1008 copy per band
                for j in range(4):
                    dst = out_sb[:, base + j * SP : base + (j + 1) * SP]
                    if not DO_MM:
                        nc.vector.memset(dst, 0.0)
                        continue
                    cands = [(t_dve + DVE_COST, "v"), (t_act + ACT_COST, "a")]
                    if GP_COST is not None:
                        cands.append((t_gp + GP_COST, "g"))
                    _, eng = min(cands)
                    if eng == "v":
                        t_dve += DVE_COST
                        nc.vector.tensor_copy(out=dst, in_=pss[j][:, 0:SP])
                    elif eng == "a":
                        t_act += ACT_COST
                        nc.scalar.copy(out=dst, in_=pss[j][:, 0:SP])
                    else:
                        t_gp += GP_COST
                        nc.gpsimd.tensor_copy(out=dst, in_=pss[j][:, 0:SP])
            elif sp == 0:
                nc.vector.memset(out_sb[:, 0:1], 0.0)

            # output store: the span is contiguous per partition in DRAM
            if sp == OUT_SPAN - 1 and DO_STORE:
                gs = g - (OUT_SPAN - 1)
                ring = getattr(
                    nc, STORE_RINGS[(g // OUT_SPAN) % len(STORE_RINGS)])
                d = OUT_SPAN * gcols // OUT_NDESC  # bf16 elems per desc
                ring.dma_start(
                    out=out_d[:, gcols * gs : gcols * (gs + OUT_SPAN)
                              ].rearrange("p (n d) -> p n d", d=d),
                    in_=out_sb[:, :].rearrange("p (n d) -> p n d", d=d))

    nc.compile()
    return nc


_NC_CACHE = {}


def _get_nc(b):
    if b not in _NC_CACHE:
        _NC_CACHE[b] = build_nc(b)
    return _NC_CACHE[b]


def make_in_maps(y, u, w):
    """Per-core input dicts for run_bass_kernel_spmd / the slope bench."""
    y = np.ascontiguousarray(np.asarray(y), dtype=np.float32)
    u = np.ascontiguousarray(np.asarray(u), dtype=np.float32)
    w = np.ascontiguousarray(np.asarray(w), dtype=np.float32)
    g32 = host_g(w)                                    # [32, S] f32
    g_rep = np.ascontiguousarray(
        np.tile(g32, (4, 1))[:, AR:].astype(NPBF16))   # [128, SP] bf16
    maps = []
    for i in range(N_CORES):
        yc, uc = y[i * B : (i + 1) * B], u[i * B : (i + 1) * B]
        X = np.concatenate(
            [yc, uc, np.ones((B, 1), np.float32)], axis=1)   # [B, 32]
        # partition q holds batch rows 64q..64q+63; tile s -> PE col q
        Xp = X.reshape(128, NTILES, K)                 # [q, s, k]
        XT = (Xp.reshape(128, GROUPS, 4, K)            # [q, g, j, k]
              .transpose(2, 3, 1, 0)                   # [j, k, g, q]
              .reshape(128, GROUPS * 128))             # row 32j+k, col 128g+q
        maps.append({"xt": np.ascontiguousarray(XT.astype(NPBF16)),
                     "g": g_rep})
    return maps


def kernel(y, u, w):
    y = np.ascontiguousarray(np.asarray(y), dtype=np.float32)
    assert y.shape == (B_FULL, AR)
    assert np.asarray(u).shape == (B_FULL, NU)
    nc = _get_nc(B)
    in_maps = make_in_maps(y, u, w)
    res = run_bass_kernel_spmd(nc, in_maps, list(range(N_CORES)))
    out = np.empty((B_FULL, S), np.float32)
    out[:, :AR] = y
    for i in range(N_CORES):
        o = np.asarray(res.results[i]["out"])   # [128, NTILES*SP] bf16
        out[i * B : (i + 1) * B, AR:] = o.reshape(B, SP)
    return out


# revision 16
# speedup vs baseline: 1.2360x; 1.2360x over previous
"""ARX recurrence kernel for Trainium2 (8 NeuronCores, data-parallel).

Math: the reference runs out[:, t] = window @ w_ar + (u @ w_u + w_b) as a
sequential scan over 1008 steps.  Since the recurrence is linear, every
output timestep is a linear functional of X = [y | u | 1]:

    out[:, t] = X @ G[:, t]          G: [32, 1024]

G depends only on the 32-element weight vector, so it is computed on the
HOST in float64 and shipped as a small bf16 input, pre-replicated across
the 4 partition quadrants.  out[:, :16] is just y, so the device only
computes/stores the 1008 predicted columns; the host splices y back in.

v2 changes vs the fp32 baseline (85.7 us) -> ~27-30 us/iter measured:
  * bf16 X^T / G inputs and bf16 output stores: the dominant output
    traffic drops 32 MB -> 15.75 MB per core.  Error ~2.4e-3 norm-rel
    vs the 2e-2 gate (PE accumulates in fp32; PSUM is fp32).
  * X^T is packed on the HOST into the exact [128, 32*ntiles] lhsT
    layout (row 32j+k, col 128g+q), removing the on-device PE
    transposes, identity matrix, and gpsimd X4 assembly.
  * Matmuls issue in band-major waves so the 4 row-tiled (K=32)
    matmuls run CONCURRENTLY on the PE 32-row sub-arrays.
  * Drains (PSUM fp32 -> SBUF bf16 cast) are one FD=1008 copy per
    band, split between DVE and ACT by a greedy balance of their
    errata-model per-instruction costs.  The steady state is
    DRAIN-bound: DVE+ACT are the only PSUM-reading engines (~1
    col/cyc each); raw store bandwidth measured ~950 GB/s (DMA_ONLY
    ablation 16.7 us), PE ~10 us, so neither binds.
  * Stores ride the SP HWDGE ring ONLY: a store's sem-wait queued on
    the ACT ring blocks ACT's subsequent drain copies behind DVE's
    progress (measured 34.4 -> 27.2 us).  SWDGE (gpsimd) stores
    measured far worse (87 us).  Store descriptor size 1-4 KB and
    OUT_SPAN made no measurable difference.

Batch <-> partition mapping: partition q holds batch rows
[64*q, 64*q + 64); batch tile s = {64q + s} maps to PE column q, so the
output store is contiguous per partition, chopped into ~4 KB
descriptors, alternating between the SP and ACT HWDGE rings.
"""

import numpy as np

import concourse.bacc as bacc
import concourse.bass as bass
import concourse.mybir as mybir
import concourse.tile as tile
from concourse.bass_utils import run_bass_kernel_spmd

N_CORES = 8
B_FULL = 65536
AR = 16          # ar order
NU = 15          # exogenous dim
K = 32           # regressor dim = AR + NU + 1
S = 1024         # sequence length
SP = S - AR      # 1008 predicted columns actually computed on device

B = B_FULL // N_CORES      # 8192 rows per core
NTILES = B // 128          # 64 batch tiles of 128 rows
GROUPS = NTILES // 4       # 16 groups of 4 tiles
F32 = mybir.dt.float32
BF16 = mybir.dt.bfloat16
NPBF16 = mybir.dt.np(BF16)

# scheduling knobs
OUT_BUFS = 4
PS_BUFS = 4             # [128,1024] fp32 tiles = 2 PSUM banks each
XT_CHUNKS = 8           # X^T loaded in chunks for pipelining
OUT_SPAN = 1            # groups per output store (1 -> ~1MB bf16)
OUT_NDESC = 2           # descriptors per partition per group-span
# stores on the SP ring ONLY: a store's sem-wait in the ACT queue blocks
# ACT's subsequent drains behind DVE's progress (measured 34.4 -> 27.2us)
STORE_RINGS = ("sync",)
DMA_ONLY = False        # ablation: only output stores (garbage data)
DO_MM = True            # ablation: skip matmuls
DO_STORE = True         # ablation: skip output stores (drain-floor probe)
# greedy drain balance: estimated per-drain cost (ns) on each engine
DVE_COST = (120 + 1008) / 0.96   # errata-adjusted PSUM->SBUF, 0.96 GHz
ACT_COST = (172 + 1008) / 1.2
GP_COST = None                   # ns/drain on gpsimd; None disables


def host_g(w):
    """Compute G [32, S] on the host in float64 (cols 0:AR = identity)."""
    w = np.asarray(w, np.float64)
    w_ar, w_u, w_b = w[:AR], w[AR : AR + NU], w[AR + NU]
    Wc = np.zeros((AR, AR + 1))
    Wc[:, :AR] = np.eye(AR)
    preds = np.empty((SP, AR + 1))
    for t in range(SP):
        pc = w_ar @ Wc
        pc[AR] += 1.0
        preds[t] = pc
        Wc = np.concatenate([Wc[1:], pc[None, :]], axis=0)
    G = np.zeros((K, S), np.float64)
    G[:AR, :AR] = np.eye(AR)
    G[:AR, AR:] = preds[:, :AR].T
    G[AR : AR + NU, AR:] = np.outer(w_u, preds[:, AR])
    G[K - 1, AR:] = w_b * preds[:, AR]
    return G.astype(np.float32)


def build_nc(b=B, reps=1):
    """Build the per-core Bass program (SPMD: same program, 8 shards).

    reps>1 unrolls the whole main loop multiple times inside one NEFF
    (writes the same outputs each rep) — used only for steady-state HW
    timing, never for grading."""
    ntiles = b // 128
    groups = ntiles // 4

    nc = bacc.Bacc("TRN2", target_bir_lowering=False, debug=False)

    xt_d = nc.dram_tensor("xt", [128, ntiles * K], BF16,
                          kind="ExternalInput").ap()
    g_d = nc.dram_tensor("g", [128, SP], BF16, kind="ExternalInput").ap()
    out_d = nc.dram_tensor("out", [128, ntiles * SP], BF16,
                           kind="ExternalOutput").ap()

    gcols = 4 * SP           # bf16 out columns per group
    n2 = SP - 512            # second matmul free dim (496)

    from contextlib import ExitStack
    with tile.TileContext(nc) as tc, ExitStack() as ctx:
        singles = ctx.enter_context(tc.tile_pool(name="singles", bufs=1))
        out_pool = ctx.enter_context(tc.tile_pool(name="outsb", bufs=OUT_BUFS))
        ps_pool = ctx.enter_context(
            tc.tile_pool(name="ps", bufs=PS_BUFS, space="PSUM"))

        # G (bf16, 1008 predicted cols), pre-replicated across quadrants.
        # Loaded in the two halves the matmul waves consume, so wave h0
        # only waits for the first 512 columns.
        G_rep = singles.tile([128, SP], BF16, tag="Grep")
        nc.sync.dma_start(out=G_rep[:, 0:512], in_=g_d[:, 0:512])
        nc.sync.dma_start(out=G_rep[:, 512:SP], in_=g_d[:, 512:SP])

        # X^T in lhsT layout, loaded in chunks so group 0 starts early.
        xt_sb = singles.tile([128, ntiles * K], BF16, tag="xt")
        nchunks = max(1, min(XT_CHUNKS, groups))
        ccols = ntiles * K // nchunks
        for c in range(nchunks):
            nc.scalar.dma_start(
                out=xt_sb[:, c * ccols : (c + 1) * ccols],
                in_=xt_d[:, c * ccols : (c + 1) * ccols])

        t_dve = t_act = t_gp = 0.0  # virtual clocks for greedy drain balance
        for g in [g for _ in range(reps) for g in range(groups)]:
            sp = g % OUT_SPAN
            if sp == 0:
                out_sb = out_pool.tile([128, OUT_SPAN * gcols], BF16,
                                       tag="outsb")
            base = sp * gcols

            if not DMA_ONLY:
                # 2 band-major waves of 4 concurrent row-tiled matmuls
                pss = []
                for j in range(4):
                    ps = ps_pool.tile([128, 1024], F32, tag="ps", name="ps")
                    pss.append(ps)
                if DO_MM:
                    for c0, nn in ((0, 512), (512, n2)):
                        for j in range(4):
                            nc.tensor.matmul(
                                pss[j][:, c0 : c0 + nn],
                                xt_sb[32 * j : 32 * (j + 1),
                                      128 * g : 128 * (g + 1)],
                                G_rep[32 * j : 32 * (j + 1), c0 : c0 + nn],
                                start=True, stop=True,
                                tile_position=(32 * j, 0),
                            )
                # drains: one fp32->bf16 FD=1008 copy per band
                drain_done = {}
                for j in range(4):
                    dst = out_sb[:, base + j * SP : base + (j + 1) * SP]
                    if not DO_MM:
                        nc.vector.memset(dst, 0.0)
                        drain_done[j] = j
                        continue
                    cands = [(t_dve + DVE_COST, "v"), (t_act + ACT_COST, "a")]
                    if GP_COST is not None:
                        cands.append((t_gp + GP_COST, "g"))
                    _, eng = min(cands)
                    if eng == "v":
                        t_dve += DVE_COST
                        drain_done[j] = t_dve
                        nc.vector.tensor_copy(out=dst, in_=pss[j][:, 0:SP])
                    elif eng == "a":
                        t_act += ACT_COST
                        drain_done[j] = t_act
                        nc.scalar.copy(out=dst, in_=pss[j][:, 0:SP])
                    else:
                        t_gp += GP_COST
                        drain_done[j] = t_gp
                        nc.gpsimd.tensor_copy(out=dst, in_=pss[j][:, 0:SP])
            elif sp == 0:
                nc.vector.memset(out_sb[:, 0:1], 0.0)

            # output store: the span is contiguous per partition in DRAM
            if sp == OUT_SPAN - 1 and DO_STORE:
                gs = g - (OUT_SPAN - 1)
                ring = getattr(
                    nc, STORE_RINGS[(g // OUT_SPAN) % len(STORE_RINGS)])
                if g == groups - 1 and OUT_SPAN == 1 and not DMA_ONLY:
                    # split the FINAL store per tile: the kernel then ends
                    # ~0.26us after the last drain instead of a full 1MB
                    # store later (tail trim on the single-run metric).
                    # Issue in drain-completion order (sync queue is FIFO).
                    for j in sorted(range(4), key=lambda j: drain_done[j]):
                        ring.dma_start(
                            out=out_d[:, gcols * gs + j * SP
                                      : gcols * gs + (j + 1) * SP
                                      ].rearrange("p (n d) -> p n d", d=SP),
                            in_=out_sb[:, j * SP : (j + 1) * SP
                                       ].rearrange("p (n d) -> p n d", d=SP))
                else:
                    d = OUT_SPAN * gcols // OUT_NDESC  # bf16 elems per desc
                    ring.dma_start(
                        out=out_d[:, gcols * gs : gcols * (gs + OUT_SPAN)
                                  ].rearrange("p (n d) -> p n d", d=d),
                        in_=out_sb[:, :].rearrange("p (n d) -> p n d", d=d))

    nc.compile()
    return nc


_NC_CACHE = {}


def _get_nc(b):
    if b not in _NC_CACHE:
        _NC_CACHE[b] = build_nc(b)
    return _NC_CACHE[b]


def make_in_maps(y, u, w):
    """Per-core input dicts for run_bass_kernel_spmd / the slope bench."""
    y = np.ascontiguousarray(np.asarray(y), dtype=np.float32)
    u = np.ascontiguousarray(np.asarray(u), dtype=np.float32)
    w = np.ascontiguousarray(np.asarray(w), dtype=np.float32)
    g32 = host_g(w)                                    # [32, S] f32
    g_rep = np.ascontiguousarray(
        np.tile(g32, (4, 1))[:, AR:].astype(NPBF16))   # [128, SP] bf16
    maps = []
    for i in range(N_CORES):
        yc, uc = y[i * B : (i + 1) * B], u[i * B : (i + 1) * B]
        X = np.concatenate(
            [yc, uc, np.ones((B, 1), np.float32)], axis=1)   # [B, 32]
        # partition q holds batch rows 64q..64q+63; tile s -> PE col q
        Xp = X.reshape(128, NTILES, K)                 # [q, s, k]
        XT = (Xp.reshape(128, GROUPS, 4, K)            # [q, g, j, k]
              .transpose(2, 3, 1, 0)                   # [j, k, g, q]
              .reshape(128, GROUPS * 128))             # row 32j+k, col 128g+q
        maps.append({"xt": np.ascontiguousarray(XT.astype(NPBF16)),
                     "g": g_rep})
    return maps


def kernel(y, u, w):
    y = np.ascontiguousarray(np.asarray(y), dtype=np.float32)
    assert y.shape == (B_FULL, AR)
    assert np.asarray(u).shape == (B_FULL, NU)
    nc = _get_nc(B)
    in_maps = make_in_maps(y, u, w)
    res = run_bass_kernel_spmd(nc, in_maps, list(range(N_CORES)))
    out = np.empty((B_FULL, S), np.float32)
    out[:, :AR] = y
    for i in range(N_CORES):
        o = np.asarray(res.results[i]["out"])   # [128, NTILES*SP] bf16
        out[i * B : (i + 1) * B, AR:] = o.reshape(B, SP)
    return out
